# revision 24
# baseline (speedup 1.0000x reference)
"""Multi-head attention Trainium2 Bass kernel (fused pipeline v2).

Problem: B=2, S=2048, D=1024, H=16, HS=64.
Sharding: tensor-parallel over heads — each of 8 cores computes 2 heads
(128 contiguous output-feature columns) for both batches; host concatenates.

Design: the exp stream on the scalar (ACT) engine is the critical resource
(~147us of ACTIVATE at N=1024).  Everything else — Q/K/V projections, the
sim and PV matmuls, PSUM evacuation — is scheduled UNDER that stream:

  * 8 groups per core: (batch, q-half, head), 16 k-chunk periods each.
    One ACT instruction [128 k, 1024 q] per period = 128 ACTs total.
  * sim matmuls (K=64) alternate PE row-halves across consecutive periods
    so they pack pairwise into disjoint row groups; this needs Q^T/K^T
    duplicated at the opposite partition half (qt2/kt2, built by DVE).
  * PV accumulation for group g runs lagged under group g+1's exp stream
    (P^T tiles buffer in SBUF), freeing PSUM banks: sims 2x[128,1024]f32
    (4 banks) + one PV accumulator [65,1024]f32 (2 banks) + projection
    scratch (2 banks) = 8 banks exactly.
  * Projections are chopped into ~1-matmul units and pumped into the PE
    stream as filler between sim/PV work, earliest-deadline-first.
  * Q/K biases ride the PSUM->SBUF cast as a DVE tensor_scalar add
    (per-partition bias); V' bias+denominator column via
    scalar_tensor_tensor with a host-broadcast bias tile.
  * The unnormalized O'^T (with denominator row 64 per head) goes to DRAM;
    the host performs the final divide and transpose during assembly.
"""

import sys
from collections import deque

sys.path.insert(0, "/opt/trn_rl_repo")

import os

import ml_dtypes
import numpy as np

import concourse.bass as bass
import concourse.mybir as mybir
import concourse.tile as tile
from concourse import bacc
from concourse import bass_utils

DEBUG_DUMP = bool(os.environ.get("K_DEBUG_DUMP"))

B, S, D = 2, 2048, 1024
H, HS = 16, 64
NCORES = 8
NTOK = B * S                  # 4096
FPC = (H // NCORES) * HS      # 128 output-feature cols per core (2 heads)
TT = 512                      # token tile for Q/K projections
NCH = D // 128                # 8 contraction chunks
QH = 1024                     # q-half width (one group's q extent)
KT = 128                      # k chunk in attention
NKT = S // KT                 # 16
VW = 2 * (HS + 1)             # 130: [V_h0 | 1 | V_h1 | 1] columns

F32 = mybir.dt.float32
BF16 = mybir.dt.bfloat16
EXP = mybir.ActivationFunctionType.Exp
ALU = mybir.AluOpType

_NC_CACHE = {}


def build_nc():
    nc = bacc.Bacc("TRN2", target_bir_lowering=False, debug=False, num_devices=NCORES)
    xt = nc.dram_tensor("xt", [D, NTOK], BF16, kind="ExternalInput").ap()
    # weights pre-arranged host-side into the SBUF layout so each loads in
    # ONE large DMA: w[p, c*F+j] = W[c*128+p, j]
    wq = nc.dram_tensor("wq", [128, NCH * FPC], BF16, kind="ExternalInput").ap()
    wk = nc.dram_tensor("wk", [128, NCH * FPC], BF16, kind="ExternalInput").ap()
    wvp = nc.dram_tensor("wvp", [128, NCH * VW], BF16, kind="ExternalInput").ap()
    bqc = nc.dram_tensor("bqc", [FPC, 1], F32, kind="ExternalInput").ap()
    bkc = nc.dram_tensor("bkc", [FPC, 1], F32, kind="ExternalInput").ap()
    bvpb = nc.dram_tensor("bvpb", [128, VW], F32, kind="ExternalInput").ap()
    out = nc.dram_tensor("out", [2 * (HS + 1), NTOK], F32, kind="ExternalOutput").ap()
    if DEBUG_DUMP:
        dbg_vp = nc.dram_tensor("dbg_vp", [128, 32 * VW], BF16, kind="ExternalOutput").ap()
        dbg_qt = nc.dram_tensor("dbg_qt", [128, NTOK], BF16, kind="ExternalOutput").ap()
        dbg_qt2 = nc.dram_tensor("dbg_qt2", [128, NTOK], BF16, kind="ExternalOutput").ap()
        dbg_kt = nc.dram_tensor("dbg_kt", [128, NTOK], BF16, kind="ExternalOutput").ap()
        dbg_kt2 = nc.dram_tensor("dbg_kt2", [128, NTOK], BF16, kind="ExternalOutput").ap()

    with tile.TileContext(nc) as tc:
        with (
            tc.tile_pool(name="persist", bufs=1) as pp,
            tc.tile_pool(name="ptp", bufs=26) as ptp,
            tc.tile_pool(name="work", bufs=2) as wkp,
            tc.tile_pool(name="psS", bufs=2, space="PSUM") as psS,
            tc.tile_pool(name="psV", bufs=1, space="PSUM") as psV,
            tc.tile_pool(name="psP", bufs=2, space="PSUM") as psP,
        ):
            # ---------------- persistent SBUF ----------------
            xtc = [pp.tile([128, NTOK], BF16, name=f"xt_{c}") for c in range(NCH)]
            wq_b = pp.tile([128, NCH * FPC], BF16)
            wk_b = pp.tile([128, NCH * FPC], BF16)
            wv_b = pp.tile([128, NCH * VW], BF16)
            bq_sb = pp.tile([128, 1], F32)
            bk_sb = pp.tile([128, 1], F32)
            bvp_sb = pp.tile([128, VW], F32)
            qt_sb = pp.tile([128, NTOK], BF16)   # Q^T: head h at partitions h*64..
            kt_sb = pp.tile([128, NTOK], BF16)   # K^T
            qt2_sb = pp.tile([128, NTOK], BF16)  # partition-swapped duplicates
            kt2_sb = pp.tile([128, NTOK], BF16)
            # V' token-major chunks, one tile per 128-token chunk so the
            # DVE-write -> PE-weight-read dependency is tracked exactly.
            vp_sb = [pp.tile([128, VW], BF16, name=f"vp_{g}") for g in range(NTOK // 128)]

            # ---------------- input DMAs ----------------
            # Weights/biases on the SWDGE (gpsimd) queue — one large DMA per
            # weight matrix (host pre-arranged) — followed by batch-1's X.
            # Batch-0's X streams on the sync queue sliced [chunk, 1024-token
            # half] so the startup Q/K projections can begin ASAP.
            nc.gpsimd.dma_start(wq_b[:], wq[:, :])
            nc.gpsimd.dma_start(wk_b[:], wk[:, :])
            nc.gpsimd.dma_start(wv_b[:], wvp[:, :])
            nc.gpsimd.dma_start(bq_sb[:], bqc[:, :])
            nc.gpsimd.dma_start(bk_sb[:], bkc[:, :])
            nc.gpsimd.dma_start(bvp_sb[:], bvpb[:, :])
            for half in range(2):  # batch 0 on sync queue
                lo = half * 1024
                for c in range(NCH):
                    nc.sync.dma_start(xtc[c][:, lo : lo + 1024], xt[c * 128 : (c + 1) * 128, lo : lo + 1024])
            for half in range(2, 4):  # batch 1 on the gpsimd queue (after weights)
                lo = half * 1024
                for c in range(NCH):
                    nc.gpsimd.dma_start(xtc[c][:, lo : lo + 1024], xt[c * 128 : (c + 1) * 128, lo : lo + 1024])

            # ---------------- projection chains (unit lists) ----------------
            def qk_chain(w_b, b_sb, dst, dst2, b, t):
                """Q/K projection of token tile t of batch b -> dst[:, tsl] (bf16)
                plus the partition-swapped duplicate in dst2. 8 matmul units."""
                tsl = slice(b * S + t * TT, b * S + (t + 1) * TT)
                cell = {}
                units = []

                def mk_mm(c):
                    def emit():
                        if c == 0:
                            cell["ps"] = psP.tile(
                                [128, TT], F32, name=f"pj_{dst.tensor.name}_{b}_{t}",
                                tag="proj", padded_shape=[128, TT],
                            )
                        nc.tensor.matmul(
                            cell["ps"][:], w_b[:, c * FPC : (c + 1) * FPC], xtc[c][:, tsl],
                            start=(c == 0), stop=(c == NCH - 1),
                        )
                    return emit

                for c in range(NCH):
                    units.append((mk_mm(c), 220))

                def cast():
                    nc.vector.tensor_scalar_add(dst[:, tsl], cell["ps"][:], b_sb[:])
                units.append((cast, 0))

                def dup_lo():
                    nc.vector.tensor_copy(dst2[64:128, tsl], dst[0:64, tsl])
                def dup_hi():
                    nc.vector.tensor_copy(dst2[0:64, tsl], dst[64:128, tsl])
                units.append((dup_lo, 0))
                units.append((dup_hi, 0))
                return units

            def v_chain(b, ch):
                """V' projection of 128-token chunk ch of batch b -> vp_sb chunk
                (token-major [128 tok, 130], denominator ones-column via bias)."""
                g = b * NKT + ch
                csl = slice(b * S + ch * 128, b * S + (ch + 1) * 128)
                cell = {}
                units = []

                def mk_mm(c):
                    def emit():
                        if c == 0:
                            cell["ps"] = psP.tile(
                                [128, VW], F32, name=f"pv_{b}_{ch}",
                                tag="proj", padded_shape=[128, TT],
                            )
                        nc.tensor.matmul(
                            cell["ps"][:], xtc[c][:, csl], wv_b[:, c * VW : (c + 1) * VW],
                            start=(c == 0), stop=(c == NCH - 1),
                        )
                    return emit

                for c in range(NCH):
                    units.append((mk_mm(c), 120))

                def cast():
                    nc.vector.scalar_tensor_tensor(
                        vp_sb[g][:], cell["ps"][:], 1.0, bvp_sb[:],
                        ALU.mult, ALU.add,
                    )
                units.append((cast, 0))
                return units

            # ---------------- filler pump ----------------
            filler = deque()

            def queue_chain(units):
                filler.extend(units)

            def pump(budget_ns):
                spent = 0
                while filler and spent < budget_ns:
                    emit, cost = filler.popleft()
                    emit()
                    spent += max(cost, 40)

            def drain_chain(units):
                for emit, _ in units:
                    emit()

            # ---------------- attention machinery ----------------
            pts = {}

            def sim_act(gi, b, qh, h, kt):
                row = 64 * ((kt + h) % 2)
                natural = row == h * 64
                ksrc = kt_sb if natural else kt2_sb
                qsrc = qt_sb if natural else qt2_sb
                koff = b * S + kt * KT
                qoff = b * S + qh * QH
                sim = psS.tile([128, QH], F32, name=f"sim_{gi}_{kt}", tag="sim")
                for qq in range(2):
                    nc.tensor.matmul(
                        sim[:, qq * 512 : (qq + 1) * 512],
                        ksrc[row : row + 64, koff : koff + KT],
                        qsrc[row : row + 64, qoff + qq * 512 : qoff + (qq + 1) * 512],
                        start=True, stop=True,
                        tile_position=(row, 0),
                    )
                pt = ptp.tile([128, QH], BF16, name=f"pt_{gi}_{kt}", tag="pt")
                nc.scalar.activation(pt[:], sim[:], EXP, scale=1.0 / np.sqrt(HS))
                pts[(gi, kt)] = pt

            pv_state = {}

            def pv_step(gi, b, qh, h, kt, pool, tag):
                """One PV accumulation step (2 matmuls) for group gi, chunk kt.
                The accumulator is two [65,512] half-tiles so it can live in
                either the dedicated psV pool or the 1-bank psP proj slots."""
                if kt == 0:
                    if pool is psV:
                        whole = pool.tile(
                            [65, QH], F32, name=f"pvp_{gi}", tag=tag,
                            padded_shape=[128, QH],
                        )
                        pv_state[gi] = [whole[:, 0:512], whole[:, 512:1024]]
                    else:
                        pv_state[gi] = [
                            pool.tile(
                                [65, 512], F32, name=f"pvp_{gi}_{qq}", tag=tag,
                                padded_shape=[128, 512],
                            )
                            for qq in range(2)
                        ]
                pvp = pv_state[gi]
                ch = b * NKT + kt
                lhsT = vp_sb[ch][:, h * (HS + 1) : (h + 1) * (HS + 1)]
                pt = pts.pop((gi, kt))
                for qq in range(2):
                    nc.tensor.matmul(
                        pvp[qq][:],
                        lhsT, pt[:, qq * 512 : (qq + 1) * 512],
                        start=(kt == 0), stop=(kt == NKT - 1),
                    )

            def extract(gi, b, qh, h):
                pvp = pv_state.pop(gi)
                ot = wkp.tile([65, QH], F32, name=f"ot_{gi}", tag="ot")
                for qq in range(2):
                    nc.vector.tensor_copy(ot[:, qq * 512 : (qq + 1) * 512], pvp[qq][:])
                nc.sync.dma_start(
                    out[h * (HS + 1) : (h + 1) * (HS + 1), b * S + qh * QH : b * S + (qh + 1) * QH],
                    ot[:],
                )

            # ---------------- schedule ----------------
            groups = [(b, qh, h) for b in range(2) for qh in range(2) for h in range(2)]
            G7 = len(groups) - 1
            NP = len(groups) * NKT  # 128 periods

            # startup: group 0's q-half and first k-chunks of projections
            drain_chain(qk_chain(wq_b, bq_sb, qt_sb, qt2_sb, 0, 0))
            drain_chain(qk_chain(wq_b, bq_sb, qt_sb, qt2_sb, 0, 1))
            drain_chain(qk_chain(wk_b, bk_sb, kt_sb, kt2_sb, 0, 0))
            drain_chain(qk_chain(wk_b, bk_sb, kt_sb, kt2_sb, 0, 1))

            # filler queue, earliest-deadline-first with ~8-period production
            # margin before each consumer (PE weight loads race ahead of the
            # matmul's semaphore wait, so just-in-time production corrupts).
            queue_chain(qk_chain(wk_b, bk_sb, kt_sb, kt2_sb, 0, 2))
            for ch in range(0, 4):
                queue_chain(v_chain(0, ch))
            queue_chain(qk_chain(wk_b, bk_sb, kt_sb, kt2_sb, 0, 3))
            for ch in range(4, NKT):
                queue_chain(v_chain(0, ch))
            for t in (2, 3):
                queue_chain(qk_chain(wq_b, bq_sb, qt_sb, qt2_sb, 0, t))
            for t in range(4):
                queue_chain(qk_chain(wq_b, bq_sb, qt_sb, qt2_sb, 1, t))
            for t in range(4):
                queue_chain(qk_chain(wk_b, bk_sb, kt_sb, kt2_sb, 1, t))
            for ch in range(NKT):
                queue_chain(v_chain(1, ch))

            # flat period loop; sims for period p+1 are emitted during period
            # p so the ACT stream never waits on freshly-issued sim matmuls.
            sim_act(0, *groups[0], 0)
            for p in range(NP):
                gi, kt = divmod(p, NKT)
                if gi > 0:
                    pv_step(gi - 1, *groups[gi - 1], kt, psV, "pvp")
                    if kt == NKT - 1:
                        extract(gi - 1, *groups[gi - 1])
                if gi == G7 and kt >= 2:
                    # last group: interleave its own PV (lag 2) in the
                    # projection-scratch banks, freed of projections by then.
                    pv_step(G7, *groups[G7], kt - 2, psP, "proj")
                pump(800 if p < 64 else 550)
                if p + 1 < NP:
                    gi2, kt2 = divmod(p + 1, NKT)
                    sim_act(gi2, *groups[gi2], kt2)
            # tail: finish PV of the last group
            pv_step(G7, *groups[G7], NKT - 2, psP, "proj")
            pv_step(G7, *groups[G7], NKT - 1, psP, "proj")
            extract(G7, *groups[G7])
            pump(10**9)
            if DEBUG_DUMP:
                for g in range(NTOK // 128):
                    nc.sync.dma_start(dbg_vp[:, g * VW : (g + 1) * VW], vp_sb[g][:])
                nc.sync.dma_start(dbg_qt[:, :], qt_sb[:, :])
                nc.sync.dma_start(dbg_qt2[:, :], qt2_sb[:, :])
                nc.sync.dma_start(dbg_kt[:, :], kt_sb[:, :])
                nc.sync.dma_start(dbg_kt2[:, :], kt2_sb[:, :])

    nc.compile()
    return nc


def get_nc():
    if "nc" not in _NC_CACHE:
        _NC_CACHE["nc"] = build_nc()
    return _NC_CACHE["nc"]


def _sbuf_layout(w):
    """[D, F] -> [128, NCH*F]: w[c*128+p, j] lands at [p, c*F+j]."""
    d, f = w.shape
    return np.ascontiguousarray(
        w.reshape(NCH, 128, f).transpose(1, 0, 2).reshape(128, NCH * f)
    )


def make_in_maps(seq_input, WQ, bQ, WK, bK, WV, bV):
    x = np.asarray(seq_input, dtype=np.float32).reshape(NTOK, D)
    xt = np.ascontiguousarray(x.T).astype(ml_dtypes.bfloat16)
    in_maps = []
    for c in range(NCORES):
        lo, hi = c * FPC, (c + 1) * FPC
        wvp = np.zeros((D, VW), dtype=np.float32)
        wvp[:, 0:HS] = WV[:, lo : lo + HS]
        wvp[:, HS + 1 : 2 * HS + 1] = WV[:, lo + HS : hi]
        bvp = np.zeros((VW,), dtype=np.float32)
        bvp[0:HS] = bV[lo : lo + HS]
        bvp[HS] = 1.0
        bvp[HS + 1 : 2 * HS + 1] = bV[lo + HS : hi]
        bvp[2 * HS + 1] = 1.0
        in_maps.append(
            {
                "xt": xt,
                "wq": _sbuf_layout(np.ascontiguousarray(WQ[:, lo:hi])).astype(ml_dtypes.bfloat16),
                "wk": _sbuf_layout(np.ascontiguousarray(WK[:, lo:hi])).astype(ml_dtypes.bfloat16),
                "wvp": _sbuf_layout(wvp).astype(ml_dtypes.bfloat16),
                "bqc": np.ascontiguousarray(bQ[lo:hi]).reshape(FPC, 1).astype(np.float32),
                "bkc": np.ascontiguousarray(bK[lo:hi]).reshape(FPC, 1).astype(np.float32),
                "bvpb": np.tile(bvp.reshape(1, VW), (128, 1)).astype(np.float32),
            }
        )
    return in_maps


def run(in_maps, trace=False):
    nc = get_nc()
    return bass_utils.run_bass_kernel_spmd(nc, in_maps, core_ids=list(range(NCORES)), trace=trace)


def kernel(seq_input, WQ, bQ, WK, bK, WV, bV):
    in_maps = make_in_maps(
        np.asarray(seq_input, np.float32),
        np.asarray(WQ, np.float32), np.asarray(bQ, np.float32),
        np.asarray(WK, np.float32), np.asarray(bK, np.float32),
        np.asarray(WV, np.float32), np.asarray(bV, np.float32),
    )
    res = run(in_maps)
    parts = []
    for c in range(NCORES):
        o = res.results[c]["out"]  # [130, 4096] feature-major, unnormalized
        for h in range(2):
            num = o[h * (HS + 1) : h * (HS + 1) + HS, :]      # [64, 4096]
            den = o[h * (HS + 1) + HS, :]                     # [4096]
            parts.append((num / den).T)                       # [4096, 64]
    full = np.concatenate(parts, axis=1)  # [4096, 1024]
    return full.reshape(B, S, H * HS)


# revision 28
# speedup vs baseline: 1.0579x; 1.0579x over previous
"""Multi-head attention Trainium2 Bass kernel (fused pipeline v2).

Problem: B=2, S=2048, D=1024, H=16, HS=64.
Sharding: tensor-parallel over heads — each of 8 cores computes 2 heads
(128 contiguous output-feature columns) for both batches; host concatenates.

Design: the exp stream on the scalar (ACT) engine is the critical resource
(~147us of ACTIVATE at N=1024).  Everything else — Q/K/V projections, the
sim and PV matmuls, PSUM evacuation — is scheduled UNDER that stream:

  * 8 groups per core: (batch, q-half, head), 16 k-chunk periods each.
    One ACT instruction [128 k, 1024 q] per period = 128 ACTs total.
  * sim matmuls (K=64) alternate PE row-halves across consecutive periods
    so they pack pairwise into disjoint row groups; this needs Q^T/K^T
    duplicated at the opposite partition half (qt2/kt2, built by DVE).
  * PV accumulation for group g runs lagged under group g+1's exp stream
    (P^T tiles buffer in SBUF), freeing PSUM banks: sims 2x[128,1024]f32
    (4 banks) + one PV accumulator [65,1024]f32 (2 banks) + projection
    scratch (2 banks) = 8 banks exactly.
  * Projections are chopped into ~1-matmul units and pumped into the PE
    stream as filler between sim/PV work, earliest-deadline-first.
  * Q/K biases ride the PSUM->SBUF cast as a DVE tensor_scalar add
    (per-partition bias); V' bias+denominator column via
    scalar_tensor_tensor with a host-broadcast bias tile.
  * The unnormalized O'^T (with denominator row 64 per head) goes to DRAM;
    the host performs the final divide and transpose during assembly.
"""

import sys
from collections import deque

sys.path.insert(0, "/opt/trn_rl_repo")

import os

import ml_dtypes
import numpy as np

import concourse.bass as bass
import concourse.mybir as mybir
import concourse.tile as tile
from concourse import bacc
from concourse import bass_utils

DEBUG_DUMP = bool(os.environ.get("K_DEBUG_DUMP"))

B, S, D = 2, 2048, 1024
H, HS = 16, 64
NCORES = 8
NTOK = B * S                  # 4096
FPC = (H // NCORES) * HS      # 128 output-feature cols per core (2 heads)
TT = 512                      # token tile for Q/K projections
NCH = D // 128                # 8 contraction chunks
QH = 1024                     # q-half width (one group's q extent)
KT = 128                      # k chunk in attention
NKT = S // KT                 # 16
VW = 2 * (HS + 1)             # 130: [V_h0 | 1 | V_h1 | 1] columns

F32 = mybir.dt.float32
BF16 = mybir.dt.bfloat16
EXP = mybir.ActivationFunctionType.Exp
ALU = mybir.AluOpType

_NC_CACHE = {}


def build_nc():
    nc = bacc.Bacc("TRN2", target_bir_lowering=False, debug=False, num_devices=NCORES)
    xt = nc.dram_tensor("xt", [D, NTOK], BF16, kind="ExternalInput").ap()
    # weights pre-arranged host-side into the SBUF layout so each loads in
    # ONE large DMA: w[p, c*F+j] = W[c*128+p, j]
    wq = nc.dram_tensor("wq", [128, NCH * FPC], BF16, kind="ExternalInput").ap()
    wk = nc.dram_tensor("wk", [128, NCH * FPC], BF16, kind="ExternalInput").ap()
    wvp = nc.dram_tensor("wvp", [128, NCH * VW], BF16, kind="ExternalInput").ap()
    bqc = nc.dram_tensor("bqc", [FPC, 1], F32, kind="ExternalInput").ap()
    bkc = nc.dram_tensor("bkc", [FPC, 1], F32, kind="ExternalInput").ap()
    bvpb = nc.dram_tensor("bvpb", [128, VW], F32, kind="ExternalInput").ap()
    out = nc.dram_tensor("out", [2 * (HS + 1), NTOK], F32, kind="ExternalOutput").ap()
    if DEBUG_DUMP:
        dbg_vp = nc.dram_tensor("dbg_vp", [128, 32 * VW], BF16, kind="ExternalOutput").ap()
        dbg_qt = nc.dram_tensor("dbg_qt", [128, NTOK], BF16, kind="ExternalOutput").ap()
        dbg_qt2 = nc.dram_tensor("dbg_qt2", [128, NTOK], BF16, kind="ExternalOutput").ap()
        dbg_kt = nc.dram_tensor("dbg_kt", [128, NTOK], BF16, kind="ExternalOutput").ap()
        dbg_kt2 = nc.dram_tensor("dbg_kt2", [128, NTOK], BF16, kind="ExternalOutput").ap()

    with tile.TileContext(nc) as tc:
        with (
            tc.tile_pool(name="persist", bufs=1) as pp,
            tc.tile_pool(name="ptp", bufs=26) as ptp,
            tc.tile_pool(name="work", bufs=2) as wkp,
            tc.tile_pool(name="psS", bufs=2, space="PSUM") as psS,
            tc.tile_pool(name="psV", bufs=1, space="PSUM") as psV,
            tc.tile_pool(name="psP", bufs=2, space="PSUM") as psP,
        ):
            # ---------------- persistent SBUF ----------------
            xtc = [pp.tile([128, NTOK], BF16, name=f"xt_{c}") for c in range(NCH)]
            wq_b = pp.tile([128, NCH * FPC], BF16)
            wk_b = pp.tile([128, NCH * FPC], BF16)
            wv_b = pp.tile([128, NCH * VW], BF16)
            bq_sb = pp.tile([128, 1], F32)
            bk_sb = pp.tile([128, 1], F32)
            bvp_sb = pp.tile([128, VW], F32)
            qt_sb = pp.tile([128, NTOK], BF16)   # Q^T: head h at partitions h*64..
            kt_sb = pp.tile([128, NTOK], BF16)   # K^T
            qt2_sb = pp.tile([128, NTOK], BF16)  # partition-swapped duplicates
            kt2_sb = pp.tile([128, NTOK], BF16)
            # V' token-major chunks, one tile per 128-token chunk so the
            # DVE-write -> PE-weight-read dependency is tracked exactly.
            vp_sb = [pp.tile([128, VW], BF16, name=f"vp_{g}") for g in range(NTOK // 128)]

            # ---------------- input DMAs ----------------
            # Weights/biases on the SWDGE (gpsimd) queue — one large DMA per
            # weight matrix (host pre-arranged) — followed by batch-1's X.
            # Batch-0's X streams on the sync queue sliced [chunk, 1024-token
            # half] so the startup Q/K projections can begin ASAP.
            nc.gpsimd.dma_start(wq_b[:], wq[:, :])
            nc.gpsimd.dma_start(wk_b[:], wk[:, :])
            nc.gpsimd.dma_start(wv_b[:], wvp[:, :])
            nc.gpsimd.dma_start(bq_sb[:], bqc[:, :])
            nc.gpsimd.dma_start(bk_sb[:], bkc[:, :])
            nc.gpsimd.dma_start(bvp_sb[:], bvpb[:, :])
            for half in range(2):  # batch 0 on sync queue
                lo = half * 1024
                for c in range(NCH):
                    nc.sync.dma_start(xtc[c][:, lo : lo + 1024], xt[c * 128 : (c + 1) * 128, lo : lo + 1024])
            # batch 1's X is deferred into the period loop (xb1_dma below) so
            # it doesn't steal HBM bandwidth from batch 0 during startup.
            xb1_slices = [
                (c, half * 1024)
                for half in range(2, 4)
                for c in range(NCH)
            ]

            def xb1_dma(n):
                for _ in range(n):
                    if xb1_slices:
                        c, lo = xb1_slices.pop(0)
                        nc.sync.dma_start(
                            xtc[c][:, lo : lo + 1024],
                            xt[c * 128 : (c + 1) * 128, lo : lo + 1024],
                        )

            # ---------------- projection chains (unit lists) ----------------
            def qk_chain(w_b, b_sb, dst, dst2, b, t):
                """Q/K projection of token tile t of batch b -> dst[:, tsl] (bf16)
                plus the partition-swapped duplicate in dst2. 8 matmul units."""
                tsl = slice(b * S + t * TT, b * S + (t + 1) * TT)
                cell = {}
                units = []

                def mk_mm(c):
                    def emit():
                        if c == 0:
                            cell["ps"] = psP.tile(
                                [128, TT], F32, name=f"pj_{dst.tensor.name}_{b}_{t}",
                                tag="proj", padded_shape=[128, TT],
                            )
                        nc.tensor.matmul(
                            cell["ps"][:], w_b[:, c * FPC : (c + 1) * FPC], xtc[c][:, tsl],
                            start=(c == 0), stop=(c == NCH - 1),
                        )
                    return emit

                for c in range(NCH):
                    units.append((mk_mm(c), 220))

                def cast():
                    nc.vector.tensor_scalar_add(dst[:, tsl], cell["ps"][:], b_sb[:])
                units.append((cast, 0))

                def dup_lo():
                    nc.vector.tensor_copy(dst2[64:128, tsl], dst[0:64, tsl])
                def dup_hi():
                    nc.vector.tensor_copy(dst2[0:64, tsl], dst[64:128, tsl])
                units.append((dup_lo, 0))
                units.append((dup_hi, 0))
                return units

            def v_chain(b, ch):
                """V' projection of 128-token chunk ch of batch b -> vp_sb chunk
                (token-major [128 tok, 130], denominator ones-column via bias)."""
                g = b * NKT + ch
                csl = slice(b * S + ch * 128, b * S + (ch + 1) * 128)
                cell = {}
                units = []

                def mk_mm(c):
                    def emit():
                        if c == 0:
                            cell["ps"] = psP.tile(
                                [128, VW], F32, name=f"pv_{b}_{ch}",
                                tag="proj", padded_shape=[128, TT],
                            )
                        nc.tensor.matmul(
                            cell["ps"][:], xtc[c][:, csl], wv_b[:, c * VW : (c + 1) * VW],
                            start=(c == 0), stop=(c == NCH - 1),
                        )
                    return emit

                for c in range(NCH):
                    units.append((mk_mm(c), 120))

                def cast():
                    nc.vector.scalar_tensor_tensor(
                        vp_sb[g][:], cell["ps"][:], 1.0, bvp_sb[:],
                        ALU.mult, ALU.add,
                    )
                units.append((cast, 0))
                return units

            # ---------------- filler pump ----------------
            filler = deque()

            def queue_chain(units):
                filler.extend(units)

            def pump(budget_ns):
                spent = 0
                while filler and spent < budget_ns:
                    emit, cost = filler.popleft()
                    emit()
                    spent += max(cost, 40)

            def drain_chain(units):
                for emit, _ in units:
                    emit()

            # ---------------- attention machinery ----------------
            pts = {}

            def sim_act(gi, b, qh, h, kt):
                row = 64 * ((kt + h) % 2)
                natural = row == h * 64
                ksrc = kt_sb if natural else kt2_sb
                qsrc = qt_sb if natural else qt2_sb
                koff = b * S + kt * KT
                qoff = b * S + qh * QH
                sim = psS.tile([128, QH], F32, name=f"sim_{gi}_{kt}", tag="sim")
                for qq in range(2):
                    nc.tensor.matmul(
                        sim[:, qq * 512 : (qq + 1) * 512],
                        ksrc[row : row + 64, koff : koff + KT],
                        qsrc[row : row + 64, qoff + qq * 512 : qoff + (qq + 1) * 512],
                        start=True, stop=True,
                        tile_position=(row, 0),
                    )
                pt = ptp.tile([128, QH], BF16, name=f"pt_{gi}_{kt}", tag="pt")
                nc.scalar.activation(pt[:], sim[:], EXP, scale=1.0 / np.sqrt(HS))
                pts[(gi, kt)] = pt

            pv_state = {}

            def pv_step(gi, b, qh, h, kt, pool, tag):
                """One PV accumulation step (2 matmuls) for group gi, chunk kt.
                The accumulator is two [65,512] half-tiles so it can live in
                either the dedicated psV pool or the 1-bank psP proj slots."""
                if kt == 0:
                    if pool is psV:
                        whole = pool.tile(
                            [65, QH], F32, name=f"pvp_{gi}", tag=tag,
                            padded_shape=[128, QH],
                        )
                        pv_state[gi] = [whole[:, 0:512], whole[:, 512:1024]]
                    else:
                        pv_state[gi] = [
                            pool.tile(
                                [65, 512], F32, name=f"pvp_{gi}_{qq}", tag=tag,
                                padded_shape=[128, 512],
                            )
                            for qq in range(2)
                        ]
                pvp = pv_state[gi]
                ch = b * NKT + kt
                lhsT = vp_sb[ch][:, h * (HS + 1) : (h + 1) * (HS + 1)]
                pt = pts.pop((gi, kt))
                for qq in range(2):
                    nc.tensor.matmul(
                        pvp[qq][:],
                        lhsT, pt[:, qq * 512 : (qq + 1) * 512],
                        start=(kt == 0), stop=(kt == NKT - 1),
                    )

            def extract(gi, b, qh, h):
                pvp = pv_state.pop(gi)
                ot = wkp.tile([65, QH], F32, name=f"ot_{gi}", tag="ot")
                for qq in range(2):
                    nc.vector.tensor_copy(ot[:, qq * 512 : (qq + 1) * 512], pvp[qq][:])
                nc.sync.dma_start(
                    out[h * (HS + 1) : (h + 1) * (HS + 1), b * S + qh * QH : b * S + (qh + 1) * QH],
                    ot[:],
                )

            # ---------------- schedule ----------------
            groups = [(b, qh, h) for b in range(2) for qh in range(2) for h in range(2)]
            G7 = len(groups) - 1
            NP = len(groups) * NKT  # 128 periods

            # pre-load the exp activation table while DMAs stream
            warm_src = pp.tile([1, 2], F32)
            warm_dst = pp.tile([1, 2], F32)
            nc.vector.memset(warm_src[:], 0.0)
            nc.scalar.activation(warm_dst[:], warm_src[:], EXP, scale=1.0)

            def drain_interleaved(ua, ub):
                """Interleave two chains' matmul units chunk-by-chunk so the
                PE stays saturated (and HAM-warm) while X chunks stream in."""
                for c in range(NCH):
                    ua[c][0]()
                    ub[c][0]()
                for u, _ in ua[NCH:]:
                    u()
                for u, _ in ub[NCH:]:
                    u()

            # startup: group 0's q-half and first k-chunks of projections
            drain_interleaved(
                qk_chain(wq_b, bq_sb, qt_sb, qt2_sb, 0, 0),
                qk_chain(wq_b, bq_sb, qt_sb, qt2_sb, 0, 1),
            )
            drain_interleaved(
                qk_chain(wk_b, bk_sb, kt_sb, kt2_sb, 0, 0),
                qk_chain(wk_b, bk_sb, kt_sb, kt2_sb, 0, 1),
            )

            # filler queue, earliest-deadline-first with ~8-period production
            # margin before each consumer (PE weight loads race ahead of the
            # matmul's semaphore wait, so just-in-time production corrupts).
            queue_chain(qk_chain(wk_b, bk_sb, kt_sb, kt2_sb, 0, 2))
            for ch in range(0, 4):
                queue_chain(v_chain(0, ch))
            queue_chain(qk_chain(wk_b, bk_sb, kt_sb, kt2_sb, 0, 3))
            for ch in range(4, NKT):
                queue_chain(v_chain(0, ch))
            for t in (2, 3):
                queue_chain(qk_chain(wq_b, bq_sb, qt_sb, qt2_sb, 0, t))
            for t in range(4):
                queue_chain(qk_chain(wq_b, bq_sb, qt_sb, qt2_sb, 1, t))
            for t in range(4):
                queue_chain(qk_chain(wk_b, bk_sb, kt_sb, kt2_sb, 1, t))
            for ch in range(NKT):
                queue_chain(v_chain(1, ch))

            # flat period loop; sims for period p+1 are emitted during period
            # p so the ACT stream never waits on freshly-issued sim matmuls.
            sim_act(0, *groups[0], 0)
            for p in range(NP):
                gi, kt = divmod(p, NKT)
                if p + 1 < NP:
                    # next period's sims lead everything else in the PE queue
                    # so filler/PV stalls never delay the ACT stream
                    gi2, kt2 = divmod(p + 1, NKT)
                    sim_act(gi2, *groups[gi2], kt2)
                if gi > 0:
                    pv_step(gi - 1, *groups[gi - 1], kt, psV, "pvp")
                    if kt == NKT - 1:
                        extract(gi - 1, *groups[gi - 1])
                if gi == G7 and kt >= 2:
                    # last group: interleave its own PV (lag 2) in the
                    # projection-scratch banks, freed of projections by then.
                    pv_step(G7, *groups[G7], kt - 2, psP, "proj")
                xb1_dma(2)
                pump(800 if p < 64 else 550)
            # tail: finish PV of the last group
            pv_step(G7, *groups[G7], NKT - 2, psP, "proj")
            pv_step(G7, *groups[G7], NKT - 1, psP, "proj")
            extract(G7, *groups[G7])
            pump(10**9)
            if DEBUG_DUMP:
                for g in range(NTOK // 128):
                    nc.sync.dma_start(dbg_vp[:, g * VW : (g + 1) * VW], vp_sb[g][:])
                nc.sync.dma_start(dbg_qt[:, :], qt_sb[:, :])
                nc.sync.dma_start(dbg_qt2[:, :], qt2_sb[:, :])
                nc.sync.dma_start(dbg_kt[:, :], kt_sb[:, :])
                nc.sync.dma_start(dbg_kt2[:, :], kt2_sb[:, :])

    nc.compile()
    return nc


def get_nc():
    if "nc" not in _NC_CACHE:
        _NC_CACHE["nc"] = build_nc()
    return _NC_CACHE["nc"]


def _sbuf_layout(w):
    """[D, F] -> [128, NCH*F]: w[c*128+p, j] lands at [p, c*F+j]."""
    d, f = w.shape
    return np.ascontiguousarray(
        w.reshape(NCH, 128, f).transpose(1, 0, 2).reshape(128, NCH * f)
    )


def make_in_maps(seq_input, WQ, bQ, WK, bK, WV, bV):
    x = np.asarray(seq_input, dtype=np.float32).reshape(NTOK, D)
    xt = np.ascontiguousarray(x.T).astype(ml_dtypes.bfloat16)
    in_maps = []
    for c in range(NCORES):
        lo, hi = c * FPC, (c + 1) * FPC
        wvp = np.zeros((D, VW), dtype=np.float32)
        wvp[:, 0:HS] = WV[:, lo : lo + HS]
        wvp[:, HS + 1 : 2 * HS + 1] = WV[:, lo + HS : hi]
        bvp = np.zeros((VW,), dtype=np.float32)
        bvp[0:HS] = bV[lo : lo + HS]
        bvp[HS] = 1.0
        bvp[HS + 1 : 2 * HS + 1] = bV[lo + HS : hi]
        bvp[2 * HS + 1] = 1.0
        in_maps.append(
            {
                "xt": xt,
                "wq": _sbuf_layout(np.ascontiguousarray(WQ[:, lo:hi])).astype(ml_dtypes.bfloat16),
                "wk": _sbuf_layout(np.ascontiguousarray(WK[:, lo:hi])).astype(ml_dtypes.bfloat16),
                "wvp": _sbuf_layout(wvp).astype(ml_dtypes.bfloat16),
                "bqc": np.ascontiguousarray(bQ[lo:hi]).reshape(FPC, 1).astype(np.float32),
                "bkc": np.ascontiguousarray(bK[lo:hi]).reshape(FPC, 1).astype(np.float32),
                "bvpb": np.tile(bvp.reshape(1, VW), (128, 1)).astype(np.float32),
            }
        )
    return in_maps


def run(in_maps, trace=False):
    nc = get_nc()
    return bass_utils.run_bass_kernel_spmd(nc, in_maps, core_ids=list(range(NCORES)), trace=trace)


def kernel(seq_input, WQ, bQ, WK, bK, WV, bV):
    in_maps = make_in_maps(
        np.asarray(seq_input, np.float32),
        np.asarray(WQ, np.float32), np.asarray(bQ, np.float32),
        np.asarray(WK, np.float32), np.asarray(bK, np.float32),
        np.asarray(WV, np.float32), np.asarray(bV, np.float32),
    )
    res = run(in_maps)
    parts = []
    for c in range(NCORES):
        o = res.results[c]["out"]  # [130, 4096] feature-major, unnormalized
        for h in range(2):
            num = o[h * (HS + 1) : h * (HS + 1) + HS, :]      # [64, 4096]
            den = o[h * (HS + 1) + HS, :]                     # [4096]
            parts.append((num / den).T)                       # [4096, 64]
    full = np.concatenate(parts, axis=1)  # [4096, 1024]
    return full.reshape(B, S, H * HS)


# revision 32
# speedup vs baseline: 1.1966x; 1.1311x over previous
"""Multi-head attention Trainium2 Bass kernel (fused pipeline v2).

Problem: B=2, S=2048, D=1024, H=16, HS=64.
Sharding: tensor-parallel over heads — each of 8 cores computes 2 heads
(128 contiguous output-feature columns) for both batches; host concatenates.

Design: the exp stream on the scalar (ACT) engine is the critical resource
(~147us of ACTIVATE at N=1024).  Everything else — Q/K/V projections, the
sim and PV matmuls, PSUM evacuation — is scheduled UNDER that stream:

  * 8 groups per core: (batch, q-half, head), 16 k-chunk periods each.
    One ACT instruction [128 k, 1024 q] per period = 128 ACTs total.
  * sim matmuls (K=64) alternate PE row-halves across consecutive periods
    so they pack pairwise into disjoint row groups; this needs Q^T/K^T
    duplicated at the opposite partition half (qt2/kt2, built by DVE).
  * PV accumulation for group g runs lagged under group g+1's exp stream
    (P^T tiles buffer in SBUF), freeing PSUM banks: sims 2x[128,1024]f32
    (4 banks) + one PV accumulator [65,1024]f32 (2 banks) + projection
    scratch (2 banks) = 8 banks exactly.
  * Projections are chopped into ~1-matmul units and pumped into the PE
    stream as filler between sim/PV work, earliest-deadline-first.
  * Q/K biases ride the PSUM->SBUF cast as a DVE tensor_scalar add
    (per-partition bias); V' bias+denominator column via
    scalar_tensor_tensor with a host-broadcast bias tile.
  * The unnormalized O'^T (with denominator row 64 per head) goes to DRAM;
    the host performs the final divide and transpose during assembly.
"""

import sys
from collections import deque

sys.path.insert(0, "/opt/trn_rl_repo")

import os

import ml_dtypes
import numpy as np

import concourse.bass as bass
import concourse.mybir as mybir
import concourse.tile as tile
from concourse import bacc
from concourse import bass_utils

DEBUG_DUMP = bool(os.environ.get("K_DEBUG_DUMP"))

B, S, D = 2, 2048, 1024
H, HS = 16, 64
NCORES = 8
NTOK = B * S                  # 4096
FPC = (H // NCORES) * HS      # 128 output-feature cols per core (2 heads)
TT = 512                      # token tile for Q/K projections
NCH = D // 128                # 8 contraction chunks
QH = 1024                     # q-half width (one group's q extent)
KT = 128                      # k chunk in attention
NKT = S // KT                 # 16
VW = 2 * (HS + 1)             # 130: [V_h0 | 1 | V_h1 | 1] columns

F32 = mybir.dt.float32
BF16 = mybir.dt.bfloat16
EXP = mybir.ActivationFunctionType.Exp
ALU = mybir.AluOpType

_NC_CACHE = {}


def build_nc():
    nc = bacc.Bacc("TRN2", target_bir_lowering=False, debug=False, num_devices=NCORES)
    xt = nc.dram_tensor("xt", [D, NTOK], BF16, kind="ExternalInput").ap()
    # weights pre-arranged host-side into the SBUF layout so each loads in
    # ONE large DMA: w[p, c*F+j] = W[c*128+p, j]
    wq = nc.dram_tensor("wq", [128, NCH * FPC], BF16, kind="ExternalInput").ap()
    wk = nc.dram_tensor("wk", [128, NCH * FPC], BF16, kind="ExternalInput").ap()
    wvp = nc.dram_tensor("wvp", [128, NCH * VW], BF16, kind="ExternalInput").ap()
    bqc = nc.dram_tensor("bqc", [FPC, 1], F32, kind="ExternalInput").ap()
    bkc = nc.dram_tensor("bkc", [FPC, 1], F32, kind="ExternalInput").ap()
    bvpb = nc.dram_tensor("bvpb", [128, VW], F32, kind="ExternalInput").ap()
    out = nc.dram_tensor("out", [2 * (HS + 1), NTOK], F32, kind="ExternalOutput").ap()
    if DEBUG_DUMP:
        dbg_vp = nc.dram_tensor("dbg_vp", [128, 32 * VW], BF16, kind="ExternalOutput").ap()
        dbg_qt = nc.dram_tensor("dbg_qt", [128, NTOK], BF16, kind="ExternalOutput").ap()
        dbg_qt2 = nc.dram_tensor("dbg_qt2", [128, NTOK], BF16, kind="ExternalOutput").ap()
        dbg_kt = nc.dram_tensor("dbg_kt", [128, NTOK], BF16, kind="ExternalOutput").ap()
        dbg_kt2 = nc.dram_tensor("dbg_kt2", [128, NTOK], BF16, kind="ExternalOutput").ap()

    with tile.TileContext(nc) as tc:
        with (
            tc.tile_pool(name="persist", bufs=1) as pp,
            tc.tile_pool(name="ptp", bufs=26) as ptp,
            tc.tile_pool(name="work", bufs=2) as wkp,
            tc.tile_pool(name="psS", bufs=2, space="PSUM") as psS,
            tc.tile_pool(name="psV", bufs=1, space="PSUM") as psV,
            tc.tile_pool(name="psP", bufs=2, space="PSUM") as psP,
        ):
            # ---------------- persistent SBUF ----------------
            xtc = [pp.tile([128, NTOK], BF16, name=f"xt_{c}") for c in range(NCH)]
            wq_b = pp.tile([128, NCH * FPC], BF16)
            wk_b = pp.tile([128, NCH * FPC], BF16)
            wv_b = pp.tile([128, NCH * VW], BF16)
            bq_sb = pp.tile([128, 1], F32)
            bk_sb = pp.tile([128, 1], F32)
            bvp_sb = pp.tile([128, VW], F32)
            qt_sb = pp.tile([128, NTOK], BF16)   # Q^T: head h at partitions h*64..
            kt_sb = pp.tile([128, NTOK], BF16)   # K^T
            qt2_sb = pp.tile([128, NTOK], BF16)  # partition-swapped duplicates
            kt2_sb = pp.tile([128, NTOK], BF16)
            # V' token-major chunks, one tile per 128-token chunk so the
            # DVE-write -> PE-weight-read dependency is tracked exactly.
            vp_sb = [pp.tile([128, VW], BF16, name=f"vp_{g}") for g in range(NTOK // 128)]

            # ---------------- input DMAs ----------------
            # Weights/biases on the SWDGE (gpsimd) queue — one large DMA per
            # weight matrix (host pre-arranged) — followed by batch-1's X.
            # Batch-0's X streams on the sync queue sliced [chunk, 1024-token
            # half] so the startup Q/K projections can begin ASAP.
            nc.gpsimd.dma_start(wq_b[:], wq[:, :])
            nc.gpsimd.dma_start(wk_b[:], wk[:, :])
            nc.gpsimd.dma_start(wv_b[:], wvp[:, :])
            nc.gpsimd.dma_start(bq_sb[:], bqc[:, :])
            nc.gpsimd.dma_start(bk_sb[:], bkc[:, :])
            nc.gpsimd.dma_start(bvp_sb[:], bvpb[:, :])
            for half in range(2):  # batch 0 on sync queue
                lo = half * 1024
                for c in range(NCH):
                    nc.sync.dma_start(xtc[c][:, lo : lo + 1024], xt[c * 128 : (c + 1) * 128, lo : lo + 1024])
            # batch 1's X is deferred into the period loop (xb1_dma below) so
            # it doesn't steal HBM bandwidth from batch 0 during startup.
            xb1_slices = [
                (c, half * 1024)
                for half in range(2, 4)
                for c in range(NCH)
            ]

            def xb1_dma(n):
                for _ in range(n):
                    if xb1_slices:
                        c, lo = xb1_slices.pop(0)
                        nc.sync.dma_start(
                            xtc[c][:, lo : lo + 1024],
                            xt[c * 128 : (c + 1) * 128, lo : lo + 1024],
                        )

            # ---------------- projection chains (unit lists) ----------------
            def qk_chain(w_b, b_sb, dst, dst2, b, t, pool=None, tag="proj"):
                """Q/K projection of token tile t of batch b -> dst[:, tsl] (bf16)
                plus the partition-swapped duplicate in dst2. 8 matmul units."""
                tsl = slice(b * S + t * TT, b * S + (t + 1) * TT)
                cell = {}
                units = []

                def mk_mm(c):
                    def emit():
                        if c == 0:
                            cell["ps"] = (pool or psP).tile(
                                [128, TT], F32, name=f"pj_{dst.tensor.name}_{b}_{t}",
                                tag=tag, padded_shape=[128, TT],
                            )
                        nc.tensor.matmul(
                            cell["ps"][:], w_b[:, c * FPC : (c + 1) * FPC], xtc[c][:, tsl],
                            start=(c == 0), stop=(c == NCH - 1),
                        )
                    return emit

                for c in range(NCH):
                    units.append((mk_mm(c), 220))

                def cast():
                    nc.vector.tensor_scalar_add(dst[:, tsl], cell["ps"][:], b_sb[:])
                units.append((cast, 0))

                def dup_lo():
                    nc.vector.tensor_copy(dst2[64:128, tsl], dst[0:64, tsl])
                def dup_hi():
                    nc.vector.tensor_copy(dst2[0:64, tsl], dst[64:128, tsl])
                units.append((dup_lo, 0))
                units.append((dup_hi, 0))
                return units

            def v_chain(b, ch):
                """V' projection of 128-token chunk ch of batch b -> vp_sb chunk
                (token-major [128 tok, 130], denominator ones-column via bias)."""
                g = b * NKT + ch
                csl = slice(b * S + ch * 128, b * S + (ch + 1) * 128)
                cell = {}
                units = []

                def mk_mm(c):
                    def emit():
                        if c == 0:
                            cell["ps"] = psP.tile(
                                [128, VW], F32, name=f"pv_{b}_{ch}",
                                tag="proj", padded_shape=[128, TT],
                            )
                        nc.tensor.matmul(
                            cell["ps"][:], xtc[c][:, csl], wv_b[:, c * VW : (c + 1) * VW],
                            start=(c == 0), stop=(c == NCH - 1),
                        )
                    return emit

                for c in range(NCH):
                    units.append((mk_mm(c), 120))

                def cast():
                    nc.vector.scalar_tensor_tensor(
                        vp_sb[g][:], cell["ps"][:], 1.0, bvp_sb[:],
                        ALU.mult, ALU.add,
                    )
                units.append((cast, 0))
                return units

            # ---------------- filler pump ----------------
            filler = deque()

            def queue_chain(units):
                filler.extend(units)

            def pump(budget_ns):
                spent = 0
                while filler and spent < budget_ns:
                    emit, cost = filler.popleft()
                    emit()
                    spent += max(cost, 40)

            def drain_chain(units):
                for emit, _ in units:
                    emit()

            # ---------------- attention machinery ----------------
            pts = {}

            def sim_act(gi, b, qh, h, kt):
                # The two q-half matmuls use opposite PE row-halves (natural
                # Q/K copy vs the partition-swapped duplicate) so they pack
                # into disjoint row groups and run concurrently.
                hp = h * 64
                koff = b * S + kt * KT
                qoff = b * S + qh * QH
                sim = psS.tile([128, QH], F32, name=f"sim_{gi}_{kt}", tag="sim")
                for qq, (row, ksrc, qsrc) in enumerate(
                    ((hp, kt_sb, qt_sb), (64 - hp, kt2_sb, qt2_sb))
                ):
                    nc.tensor.matmul(
                        sim[:, qq * 512 : (qq + 1) * 512],
                        ksrc[row : row + 64, koff : koff + KT],
                        qsrc[row : row + 64, qoff + qq * 512 : qoff + (qq + 1) * 512],
                        start=True, stop=True,
                        tile_position=(row, 0),
                    )
                pt = ptp.tile([128, QH], BF16, name=f"pt_{gi}_{kt}", tag="pt")
                nc.scalar.activation(pt[:], sim[:], EXP, scale=1.0 / np.sqrt(HS))
                pts[(gi, kt)] = pt

            pv_state = {}

            def pv_step(gi, b, qh, h, kt, pool, tag):
                """One PV accumulation step (2 matmuls) for group gi, chunk kt.
                The accumulator is two [65,512] half-tiles so it can live in
                either the dedicated psV pool or the 1-bank psP proj slots."""
                if kt == 0:
                    if pool is psV:
                        whole = pool.tile(
                            [65, QH], F32, name=f"pvp_{gi}", tag=tag,
                            padded_shape=[128, QH],
                        )
                        pv_state[gi] = [whole[:, 0:512], whole[:, 512:1024]]
                    else:
                        pv_state[gi] = [
                            pool.tile(
                                [65, 512], F32, name=f"pvp_{gi}_{qq}", tag=tag,
                                padded_shape=[128, 512],
                            )
                            for qq in range(2)
                        ]
                pvp = pv_state[gi]
                ch = b * NKT + kt
                lhsT = vp_sb[ch][:, h * (HS + 1) : (h + 1) * (HS + 1)]
                pt = pts.pop((gi, kt))
                for qq in range(2):
                    nc.tensor.matmul(
                        pvp[qq][:],
                        lhsT, pt[:, qq * 512 : (qq + 1) * 512],
                        start=(kt == 0), stop=(kt == NKT - 1),
                    )

            def extract(gi, b, qh, h):
                pvp = pv_state.pop(gi)
                ot = wkp.tile([65, QH], F32, name=f"ot_{gi}", tag="ot")
                for qq in range(2):
                    nc.vector.tensor_copy(ot[:, qq * 512 : (qq + 1) * 512], pvp[qq][:])
                nc.sync.dma_start(
                    out[h * (HS + 1) : (h + 1) * (HS + 1), b * S + qh * QH : b * S + (qh + 1) * QH],
                    ot[:],
                )

            # ---------------- schedule ----------------
            groups = [(b, qh, h) for b in range(2) for qh in range(2) for h in range(2)]
            G7 = len(groups) - 1
            NP = len(groups) * NKT  # 128 periods

            # pre-load the exp activation table while DMAs stream
            warm_src = pp.tile([1, 2], F32)
            warm_dst = pp.tile([1, 2], F32)
            nc.vector.memset(warm_src[:], 0.0)
            nc.scalar.activation(warm_dst[:], warm_src[:], EXP, scale=1.0)

            def drain_interleaved(*chains):
                """Interleave chains' matmul units chunk-by-chunk so the PE
                stays saturated (and HAM-warm) while X chunks stream in."""
                for c in range(NCH):
                    for u in chains:
                        u[c][0]()
                for u in chains:
                    for f, _ in u[NCH:]:
                        f()

            # startup: group 0's q-half + first K tiles; K chains borrow the
            # (still idle) sim PSUM slots so all four run concurrently.
            drain_interleaved(
                qk_chain(wq_b, bq_sb, qt_sb, qt2_sb, 0, 0),
                qk_chain(wq_b, bq_sb, qt_sb, qt2_sb, 0, 1),
                qk_chain(wk_b, bk_sb, kt_sb, kt2_sb, 0, 0, pool=psS, tag="sim"),
                qk_chain(wk_b, bk_sb, kt_sb, kt2_sb, 0, 1, pool=psS, tag="sim"),
            )
            for ch in range(4):
                drain_chain(v_chain(0, ch))

            # filler queue, earliest-deadline-first with ~8-period production
            # margin before each consumer (PE weight loads race ahead of the
            # matmul's semaphore wait, so just-in-time production corrupts).
            queue_chain(qk_chain(wk_b, bk_sb, kt_sb, kt2_sb, 0, 2))
            queue_chain(qk_chain(wk_b, bk_sb, kt_sb, kt2_sb, 0, 3))
            for ch in range(4, NKT):
                queue_chain(v_chain(0, ch))
            for t in (2, 3):
                queue_chain(qk_chain(wq_b, bq_sb, qt_sb, qt2_sb, 0, t))
            for t in range(4):
                queue_chain(qk_chain(wq_b, bq_sb, qt_sb, qt2_sb, 1, t))
                queue_chain(qk_chain(wk_b, bk_sb, kt_sb, kt2_sb, 1, t))
            for ch in range(NKT):
                queue_chain(v_chain(1, ch))

            # flat period loop; sims for period p+1 are emitted during period
            # p so the ACT stream never waits on freshly-issued sim matmuls.
            sim_act(0, *groups[0], 0)
            for p in range(NP):
                gi, kt = divmod(p, NKT)
                if p + 1 < NP:
                    # next period's sims lead everything else in the PE queue
                    # so filler/PV stalls never delay the ACT stream
                    gi2, kt2 = divmod(p + 1, NKT)
                    sim_act(gi2, *groups[gi2], kt2)
                if gi > 0:
                    pv_step(gi - 1, *groups[gi - 1], kt, psV, "pvp")
                    if kt == NKT - 1:
                        extract(gi - 1, *groups[gi - 1])
                if gi == G7 and kt >= 2:
                    # last group: interleave its own PV (lag 2) in the
                    # projection-scratch banks, freed of projections by then.
                    pv_step(G7, *groups[G7], kt - 2, psP, "proj")
                xb1_dma(2)
                pump(800 if p < 16 else 480)
            # tail: finish PV of the last group
            pv_step(G7, *groups[G7], NKT - 2, psP, "proj")
            pv_step(G7, *groups[G7], NKT - 1, psP, "proj")
            extract(G7, *groups[G7])
            pump(10**9)
            if DEBUG_DUMP:
                for g in range(NTOK // 128):
                    nc.sync.dma_start(dbg_vp[:, g * VW : (g + 1) * VW], vp_sb[g][:])
                nc.sync.dma_start(dbg_qt[:, :], qt_sb[:, :])
                nc.sync.dma_start(dbg_qt2[:, :], qt2_sb[:, :])
                nc.sync.dma_start(dbg_kt[:, :], kt_sb[:, :])
                nc.sync.dma_start(dbg_kt2[:, :], kt2_sb[:, :])

    nc.compile()
    return nc


def get_nc():
    if "nc" not in _NC_CACHE:
        _NC_CACHE["nc"] = build_nc()
    return _NC_CACHE["nc"]


def _sbuf_layout(w):
    """[D, F] -> [128, NCH*F]: w[c*128+p, j] lands at [p, c*F+j]."""
    d, f = w.shape
    return np.ascontiguousarray(
        w.reshape(NCH, 128, f).transpose(1, 0, 2).reshape(128, NCH * f)
    )


def make_in_maps(seq_input, WQ, bQ, WK, bK, WV, bV):
    x = np.asarray(seq_input, dtype=np.float32).reshape(NTOK, D)
    xt = np.ascontiguousarray(x.T).astype(ml_dtypes.bfloat16)
    in_maps = []
    for c in range(NCORES):
        lo, hi = c * FPC, (c + 1) * FPC
        wvp = np.zeros((D, VW), dtype=np.float32)
        wvp[:, 0:HS] = WV[:, lo : lo + HS]
        wvp[:, HS + 1 : 2 * HS + 1] = WV[:, lo + HS : hi]
        bvp = np.zeros((VW,), dtype=np.float32)
        bvp[0:HS] = bV[lo : lo + HS]
        bvp[HS] = 1.0
        bvp[HS + 1 : 2 * HS + 1] = bV[lo + HS : hi]
        bvp[2 * HS + 1] = 1.0
        in_maps.append(
            {
                "xt": xt,
                "wq": _sbuf_layout(np.ascontiguousarray(WQ[:, lo:hi])).astype(ml_dtypes.bfloat16),
                "wk": _sbuf_layout(np.ascontiguousarray(WK[:, lo:hi])).astype(ml_dtypes.bfloat16),
                "wvp": _sbuf_layout(wvp).astype(ml_dtypes.bfloat16),
                "bqc": np.ascontiguousarray(bQ[lo:hi]).reshape(FPC, 1).astype(np.float32),
                "bkc": np.ascontiguousarray(bK[lo:hi]).reshape(FPC, 1).astype(np.float32),
                "bvpb": np.tile(bvp.reshape(1, VW), (128, 1)).astype(np.float32),
            }
        )
    return in_maps


def run(in_maps, trace=False):
    nc = get_nc()
    return bass_utils.run_bass_kernel_spmd(nc, in_maps, core_ids=list(range(NCORES)), trace=trace)


def kernel(seq_input, WQ, bQ, WK, bK, WV, bV):
    in_maps = make_in_maps(
        np.asarray(seq_input, np.float32),
        np.asarray(WQ, np.float32), np.asarray(bQ, np.float32),
        np.asarray(WK, np.float32), np.asarray(bK, np.float32),
        np.asarray(WV, np.float32), np.asarray(bV, np.float32),
    )
    res = run(in_maps)
    parts = []
    for c in range(NCORES):
        o = res.results[c]["out"]  # [130, 4096] feature-major, unnormalized
        for h in range(2):
            num = o[h * (HS + 1) : h * (HS + 1) + HS, :]      # [64, 4096]
            den = o[h * (HS + 1) + HS, :]                     # [4096]
            parts.append((num / den).T)                       # [4096, 64]
    full = np.concatenate(parts, axis=1)  # [4096, 1024]
    return full.reshape(B, S, H * HS)
